# revision 17
# baseline (speedup 1.0000x reference)
"""Causal self-attention with ALiBi on 8 trn2 cores.

Sharding: data-parallel over batch (2) x tensor-parallel over head groups (4).
Core c handles batch b = c // 4, head group g = c % 4 (heads 4g..4g+3).

v2 design notes (all timings vs the 261us baseline trace):
- bf16 everywhere on the PE except the x operand (kept f32r, exact):
  weights come from the host as bf16 (halves weight DMA), q/k/v/ex/y are
  written back as bf16 (halves SBUF traffic, enables FWL weight loads).
- ALiBi stays EXACT in bf16 via a 3-row augmented contraction:
  k_aug = [j_hi; j_lo; 1], q_aug = [slope; slope; -slope*i] with
  j = j_hi + j_lo, j_hi multiple of 128 (4-bit mantissa), j_lo < 128
  (7 bits): both products are exact; the -slope*i rounding is constant
  per query column so softmax cancels it exactly.
- ALiBi underflow chunk-skipping: head-slot h (slope >= (h+1)/16 on every
  core) gives exp(score) == 0 in f32 whenever slope*dist > 110, so those
  score/exp/AV chunks are skipped outright (~29% of full chunks).
- HAM warm-up: the PE clock-gates to 1.2 GHz until ~3.4us of sustained
  activity; dummy matmuls on the identity tile warm it during the initial
  x DMA, and the pair-interleaved attention keeps it dense afterwards.
- Attention interleaves the two heads of a pair chunk-by-chunk so the
  exp (ACT engine) latency of one head hides under the other head's
  score/AV matmuls; the output projection of the previous query block is
  emitted between pair0 and pair1 to cover pair0's softmax-normalize.
- Softmax normalize: denominators come out of the AV matmul via a ones
  column in v (row 64 for even heads, 32 for odd); reciprocal_approx_fast
  (single DVE op) + SBUF->SBUF broadcast DMA replace the 3.3us
  InstReciprocal + DRAM round-trip of the baseline.
"""

import sys

sys.path.insert(0, "/opt/trn_rl_repo")

import numpy as np

import concourse.bacc as bacc
import concourse.mybir as mybir
import concourse.tile as tile
from concourse.bass import ds, ts
from concourse.bass_utils import run_bass_kernel_spmd

B, T, D, H, DH = 2, 2048, 1024, 16, 64
G = 4            # head groups (tensor-parallel)
HPC = H // G     # heads per core
DG = D // G      # model dims per core (256)
P = 128
N_CORES = 8
NEG = -1.0e30

F32 = mybir.dt.float32
F32R = mybir.dt.float32r
BF16 = mybir.dt.bfloat16
ADD = mybir.AluOpType.add
MULT = mybir.AluOpType.mult
EXP = mybir.ActivationFunctionType.Exp

TRACE = False
LAST_RESULTS = None

_cache = {}

# ALiBi skip thresholds per head slot: slope >= (slot+1)/16 on every core,
# so dist > 110*16/(slot+1) guarantees exp underflows to 0 in f32.
DIST_MAX = [110 * 16 // (h + 1) for h in range(HPC)]   # [1760, 880, 586, 440]


def _kept_jcs(h, qb):
    """Key chunks contributing to query block qb for head slot h."""
    out = []
    for jc in range(qb * 4 + 4):
        if jc < qb * 4:  # non-diagonal: min distance query->key
            dist = qb * 512 - (jc * P + P - 1)
            if dist > DIST_MAX[h]:
                continue
        out.append(jc)
    return out


def _build(with_bias: bool):
    nc = bacc.Bacc("TRN2", target_bir_lowering=False, debug=False)

    xT_d = nc.dram_tensor("xT", [D, T], F32, kind="ExternalInput").ap()
    wq_d = nc.dram_tensor("wqT", [D, DG], BF16, kind="ExternalInput").ap()
    wk_d = nc.dram_tensor("wkT", [D, DG], BF16, kind="ExternalInput").ap()
    wv_d = nc.dram_tensor("wvT", [D, DG], BF16, kind="ExternalInput").ap()
    wo_d = nc.dram_tensor("woT", [DG, D], BF16, kind="ExternalInput").ap()
    qaug_d = nc.dram_tensor("qaug", [HPC, 3, T], BF16, kind="ExternalInput").ap()
    kaug_d = nc.dram_tensor("kaug", [3, T], BF16, kind="ExternalInput").ap()
    ident_d = nc.dram_tensor("ident", [P, P], BF16, kind="ExternalInput").ap()
    maskst_d = nc.dram_tensor("maskst", [P, P], BF16, kind="ExternalInput").ap()
    if with_bias:
        bvo_d = nc.dram_tensor("bvo", [P, DG], F32, kind="ExternalInput").ap()
        bq_d = nc.dram_tensor("bq2", [P, 2], F32, kind="ExternalInput").ap()
        bk_d = nc.dram_tensor("bk2", [P, 2], F32, kind="ExternalInput").ap()
    out_d = nc.dram_tensor("outT", [D, T], BF16, kind="ExternalOutput").ap()
    dscr_d = nc.dram_tensor("dscratch", [16, 512], F32).ap()

    with tile.TileContext(nc) as tc:
        with (
            tc.tile_pool(name="big", bufs=1) as big,
            tc.tile_pool(name="xtp", bufs=2) as xtp,
            tc.tile_pool(name="stage", bufs=3) as stage,
            tc.tile_pool(name="expp", bufs=3) as expp,
            tc.tile_pool(name="small", bufs=2) as small,
            tc.tile_pool(name="acc", bufs=2, space="PSUM") as accp,
            tc.tile_pool(name="pss", bufs=2, space="PSUM") as pssp,
            tc.tile_pool(name="pyy", bufs=1, space="PSUM") as pyp,
        ):
            # ---- persistent tiles
            wv = [big.tile([P, DG], BF16, tag=f"wv{i}", name=f"wv{i}") for i in range(8)]
            wq = [big.tile([P, DG], BF16, tag=f"wq{i}", name=f"wq{i}") for i in range(8)]
            wk = [big.tile([P, DG], BF16, tag=f"wk{i}", name=f"wk{i}") for i in range(8)]
            wo = [big.tile([P, D], BF16, tag=f"wo{i}", name=f"wo{i}") for i in range(2)]
            qa = [big.tile([67, T], BF16, tag=f"qa{h}", name=f"qa{h}") for h in range(HPC)]
            ka = [big.tile([67, T], BF16, tag=f"ka{h}", name=f"ka{h}") for h in range(HPC)]
            # va layout: per head block of 128 cols; even head: v at +0:64,
            # ones at +64 (AV out rows 0:65); odd head: ones at +32, v at
            # +64:128 (AV out rows 0:128, y at 64:128, denom at 32).
            va = big.tile([P, 16, 4 * P], BF16, tag="va", name="va")
            yt = [big.tile([P, T], BF16, tag=f"yt{m}", name=f"yt{m}") for m in range(2)]
            dn = [big.tile([64, 512], F32, tag=f"dn{m}", name=f"dn{m}") for m in range(2)]
            ident_sb = big.tile([P, P], BF16, tag="ident")
            maskst_sb = big.tile([P, P], BF16, tag="maskst")
            if with_bias:
                bvo = big.tile([P, DG], F32, tag="bvo")
                bq2 = big.tile([P, 2], F32, tag="bq2")
                bk2 = big.tile([P, 2], F32, tag="bk2")

            # ---- loads.  ident first (feeds the HAM warm-up dummies), then
            # the t0 x block in column-quarters so vproj can start early,
            # weights on the scalar queue, remaining x blocks on gpsimd.
            nc.sync.dma_start(out=ident_sb[:], in_=ident_d[:])
            nc.sync.dma_start(out=maskst_sb[:], in_=maskst_d[:])
            for h in range(HPC):
                nc.sync.dma_start(out=qa[h][64:67, :], in_=qaug_d[h])
                nc.sync.dma_start(out=ka[h][64:67, :], in_=kaug_d[:])
            if with_bias:
                nc.sync.dma_start(out=bvo[:], in_=bvo_d[:])
                nc.sync.dma_start(out=bq2[:], in_=bq_d[:])
                nc.sync.dma_start(out=bk2[:], in_=bk_d[:])

            for i in range(8):
                nc.sync.dma_start(out=wv[i][:], in_=wv_d[ts(i, P), :])

            xtile = [[None] * 8 for _ in range(4)]
            # x loads all go through the gpsimd queue: it is the only one
            # that can cast f32 -> bf16 in flight.  t0 in column-quarters,
            # chunk-major, so vproj chunk ch can start early.
            for i in range(8):
                xtile[0][i] = xtp.tile([P, 512], BF16, tag=f"xt{i}", name=f"x0_{i}")
            for ch in range(4):
                for i in range(8):
                    nc.gpsimd.dma_start(
                        out=xtile[0][i][:, ts(ch, P)],
                        in_=xT_d[ts(i, P), ds(ch * P, P)],
                    )
            for i in range(8):
                nc.sync.dma_start(out=wq[i][:], in_=wq_d[ts(i, P), :])
            for i in range(8):
                nc.sync.dma_start(out=wk[i][:], in_=wk_d[ts(i, P), :])
            for tq in range(1, 4):
                for i in range(8):
                    t_ = xtp.tile([P, 512], BF16, tag=f"xt{i}", name=f"x{tq}_{i}")
                    nc.gpsimd.dma_start(out=t_[:], in_=xT_d[ts(i, P), ts(tq, 512)])
                    xtile[tq][i] = t_
            for i in range(2):
                nc.sync.dma_start(out=wo[i][:], in_=wo_d[ts(i, P), :])

            # va constants: even h ones at h*128+64, odd h junk cols
            # h*128+0:64 zeroed then ones at h*128+32.
            nc.vector.memset(va[:, :, 64:65], 1.0)                    # h0 ones
            nc.vector.memset(va[:, :, 2 * P + 64 : 2 * P + 65], 1.0)  # h2 ones
            nc.vector.memset(va[:, :, P : P + 64], 0.0)               # h1 zeros
            nc.vector.memset(va[:, :, P + 32 : P + 33], 1.0)          # h1 ones
            nc.vector.memset(va[:, :, 3 * P : 3 * P + 64], 0.0)       # h3 zeros
            nc.vector.memset(va[:, :, 3 * P + 32 : 3 * P + 33], 1.0)  # h3 ones
            # denominators scratch: keep finite so reciprocal_approx_fast
            # never sees uninitialized garbage in its unused rows.
            nc.vector.memset(dn[0][:], 1.0)
            nc.vector.memset(dn[1][:], 1.0)

            # ---- HAM warm-up: dummy matmuls while x streams in (junk data,
            # 512-wide moving so each burns ~200-430ns of PE activity).
            junk = big.tile([P, 512], BF16, tag="junk", name="junk")
            nc.vector.memset(junk[:], 0.0)
            for w in range(20):
                pdum = accp.tile([P, 512], F32, tag="acc", name=f"dum{w}")
                nc.tensor.matmul(
                    out=pdum[:], lhsT=ident_sb[:], rhs=junk[:],
                    start=True, stop=True,
                )

            # ---- projections for one 512-wide t block
            def emit_proj(tq):
                xb = xtile[tq]
                # v: [t,dv] chunks; moving = wv (bf16, 256 cols)
                for ch in range(4):
                    tch = tq * 4 + ch
                    pv = accp.tile([P, DG], F32, tag="acc", name=f"pv{tch}")
                    for kc in range(8):
                        nc.tensor.matmul(
                            out=pv[:],
                            lhsT=xb[kc][:, ts(ch, P)],
                            rhs=wv[kc][:],
                            start=(kc == 0),
                            stop=(kc == 7),
                        )
                    # scatter into va: even heads v at cols h*128+0:64,
                    # odd heads v at cols h*128+64:128.  One strided op per
                    # parity via (2, 256)-block views.
                    vdst = va[:, tch, :].rearrange("p (b c) -> p b c", b=2)
                    vsrc = pv[:].rearrange("p (b c) -> p b c", b=2)
                    if with_bias:
                        bsrc = bvo[:].rearrange("p (b c) -> p b c", b=2)
                        nc.vector.tensor_tensor(
                            out=vdst[:, :, 0:64],
                            in0=vsrc[:, :, 0:64],
                            in1=bsrc[:, :, 0:64],
                            op=ADD,
                        )
                        nc.vector.tensor_tensor(
                            out=vdst[:, :, 192:256],
                            in0=vsrc[:, :, 64:128],
                            in1=bsrc[:, :, 64:128],
                            op=ADD,
                        )
                    else:
                        nc.vector.tensor_copy(
                            out=vdst[:, :, 0:64], in_=vsrc[:, :, 0:64]
                        )
                        nc.vector.tensor_copy(
                            out=vdst[:, :, 192:256], in_=vsrc[:, :, 64:128]
                        )
                # q/k: [d',t] via bf16 weight stationaries
                for wt, dst, bias_name in ((wq, qa, "q"), (wk, ka, "k")):
                    bt = (bq2 if bias_name == "q" else bk2) if with_bias else None
                    for mc in range(2):
                        pq = accp.tile([P, 512], F32, tag="acc", name=f"p{bias_name}{tq}_{mc}")
                        for kc in range(8):
                            nc.tensor.matmul(
                                out=pq[:],
                                lhsT=wt[kc][:, ts(mc, P)],
                                rhs=xb[kc][:],
                                start=(kc == 0),
                                stop=(kc == 7),
                            )
                        h_even, h_odd = 2 * mc, 2 * mc + 1
                        if with_bias:
                            nc.vector.tensor_scalar(
                                out=dst[h_even][0:64, ts(tq, 512)],
                                in0=pq[0:64, :],
                                scalar1=bt[0:64, mc : mc + 1],
                                scalar2=None,
                                op0=ADD,
                            )
                            nc.vector.tensor_scalar(
                                out=dst[h_odd][0:64, ts(tq, 512)],
                                in0=pq[64:128, :],
                                scalar1=bt[64:128, mc : mc + 1],
                                scalar2=None,
                                op0=ADD,
                            )
                        else:
                            nc.vector.tensor_copy(
                                out=dst[h_even][0:64, ts(tq, 512)], in_=pq[0:64, :]
                            )
                            nc.vector.tensor_copy(
                                out=dst[h_odd][0:64, ts(tq, 512)], in_=pq[64:128, :]
                            )

            # ---- causal flash attention, pair-interleaved
            def emit_attention_pair(qb, pair):
                o = qb * 512
                hs = (2 * pair, 2 * pair + 1)
                jlists = {h: _kept_jcs(h, qb) for h in hs}
                pys = {}
                for h in hs:
                    py = pyp.tile([P, 512], F32, tag=f"py{h % 2}", name=f"py{qb}_{h}")
                    pys[h] = py
                pend = {h: None for h in hs}
                first_av = {h: True for h in hs}
                all_jcs = sorted(set(jlists[hs[0]]) | set(jlists[hs[1]]))
                for jc in all_jcs:
                    for h in hs:
                        if jc not in jlists[h]:
                            continue
                        r = jc * P - o
                        lo = max(r, 0)
                        ps = pssp.tile(
                            [P, 512], F32, tag=f"ps{h % 2}", name=f"ps{qb}_{h}_{jc}"
                        )
                        if r < 0:
                            nc.tensor.matmul(
                                out=ps[:],
                                lhsT=ka[h][0:67, ts(jc, P)],
                                rhs=qa[h][0:67, ds(o, 512)],
                                start=True,
                                stop=True,
                            )
                        else:
                            nc.tensor.matmul(
                                out=ps[:, lo:512],
                                lhsT=ka[h][0:67, ts(jc, P)],
                                rhs=qa[h][0:67, ds(o + lo, 512 - lo)],
                                start=True,
                                stop=False,
                            )
                            nc.tensor.matmul(
                                out=ps[:, lo : lo + P],
                                lhsT=ident_sb[:],
                                rhs=maskst_sb[:],
                                start=False,
                                stop=True,
                            )
                        ex = expp.tile(
                            [P, 512], BF16, tag=f"ex{h % 2}", name=f"ex{qb}_{h}_{jc}"
                        )
                        nc.scalar.activation(
                            out=ex[:, lo:512], in_=ps[:, lo:512], func=EXP
                        )
                        if pend[h] is not None:
                            pjc, plo, pex = pend[h]
                            _emit_av(
                                nc, pys[h], va, h, pjc, plo, pex, first_av[h], False
                            )
                            first_av[h] = False
                        pend[h] = (jc, lo, ex)
                for h in hs:
                    pjc, plo, pex = pend[h]
                    _emit_av(nc, pys[h], va, h, pjc, plo, pex, first_av[h], True)

                # softmax denominators -> reciprocal -> broadcast -> scale
                h0, h1 = hs
                dnt = dn[pair]
                nc.vector.tensor_copy(out=dnt[0:1, :], in_=pys[h0][64:65, :])
                nc.vector.tensor_copy(out=dnt[32:33, :], in_=pys[h1][32:33, :])
                dn2 = small.tile([64, 512], F32, tag="dn2", name=f"dn2{qb}_{pair}")
                nc.vector.reciprocal_approx_fast(out=dn2[0:33, :], in_=dnt[0:33, :])
                rb = small.tile([P, 512], F32, tag="rb", name=f"rb{qb}_{pair}")
                idx = qb * 2 + pair
                nc.sync.dma_start(out=dscr_d[idx : idx + 1, :], in_=dn2[0:1, :])
                nc.gpsimd.dma_start(
                    out=dscr_d[8 + idx : 9 + idx, :], in_=dn2[32:33, :]
                )
                nc.sync.dma_start(
                    out=rb[0:64, :],
                    in_=dscr_d[idx : idx + 1, :].to_broadcast((64, 512)),
                )
                nc.gpsimd.dma_start(
                    out=rb[64:128, :],
                    in_=dscr_d[8 + idx : 9 + idx, :].to_broadcast((64, 512)),
                )
                nc.vector.tensor_tensor(
                    out=yt[pair][0:64, ds(o, 512)],
                    in0=pys[h0][0:64, :],
                    in1=rb[0:64, :],
                    op=MULT,
                )
                nc.vector.tensor_tensor(
                    out=yt[pair][64:128, ds(o, 512)],
                    in0=pys[h1][64:128, :],
                    in1=rb[64:128, :],
                    op=MULT,
                )

            def emit_outproj(qb):
                for ec in range(8):
                    po = accp.tile([P, 512], F32, tag="acc", name=f"po{qb}_{ec}")
                    for k2 in range(2):
                        nc.tensor.matmul(
                            out=po[:],
                            lhsT=wo[k2][:, ts(ec, P)],
                            rhs=yt[k2][:, ts(qb, 512)],
                            start=(k2 == 0),
                            stop=(k2 == 1),
                        )
                    ob = stage.tile([P, 512], BF16, tag="stage", name="ob")
                    nc.vector.tensor_copy(out=ob[:], in_=po[:])
                    eng = nc.sync if ec % 2 == 0 else nc.gpsimd
                    eng.dma_start(out=out_d[ts(ec, P), ts(qb, 512)], in_=ob[:])

            for tq in range(4):
                emit_proj(tq)
                emit_attention_pair(tq, 0)
                if tq > 0:
                    emit_outproj(tq - 1)
                emit_attention_pair(tq, 1)
            emit_outproj(3)

    nc.compile()
    return nc


def _va_cols(h):
    # AV stationary slice for head h within its 128-col va block
    if h % 2 == 0:
        return slice(h * P, h * P + 65)
    return slice(h * P, h * P + P)


def _emit_av(nc, py, va, h, pjc, plo, pex, start, stop):
    cols = _va_cols(h)
    rows = 65 if h % 2 == 0 else P
    nc.tensor.matmul(
        out=py[0:rows, plo:512],
        lhsT=va[:, pjc, cols],
        rhs=pex[:, plo:512],
        start=start,
        stop=stop,
    )


def _get_nc(with_bias: bool):
    if with_bias not in _cache:
        _cache[with_bias] = _build(with_bias)
    return _cache[with_bias]


def kernel(x, freqs_cis, Wq, bq, Wkv, bkv, Wo, bo, **_unused):
    import ml_dtypes

    x = np.asarray(x, np.float32)
    Wq = np.asarray(Wq, np.float32)
    bq = np.asarray(bq, np.float32)
    Wkv = np.asarray(Wkv, np.float32)
    bkv = np.asarray(bkv, np.float32)
    Wo = np.asarray(Wo, np.float32)
    bo = np.asarray(bo, np.float32)

    with_bias = bool(np.any(bq) or np.any(bkv))
    nc = _get_nc(with_bias)

    scale = 1.0 / np.sqrt(DH)
    iota = np.arange(T, dtype=np.float32)

    mm = np.arange(P, dtype=np.float32)
    maskst = np.where(mm[None, :] < mm[:, None], NEG, 0.0).astype(ml_dtypes.bfloat16)
    ident = np.eye(P, dtype=ml_dtypes.bfloat16)

    j_hi = np.floor(iota / P) * P
    j_lo = iota - j_hi
    kaug = np.stack([j_hi, j_lo, np.ones(T, np.float32)]).astype(ml_dtypes.bfloat16)

    xT = [np.ascontiguousarray(x[b].T) for b in range(B)]  # [D, T]

    in_maps = []
    for c in range(N_CORES):
        b, g = divmod(c, G)
        rows = slice(g * DG, (g + 1) * DG)
        wqT = np.ascontiguousarray((Wq[rows] * scale).T).astype(ml_dtypes.bfloat16)
        wkT = np.ascontiguousarray(Wkv[0:D][rows].T).astype(ml_dtypes.bfloat16)
        wvT = np.ascontiguousarray(Wkv[D : 2 * D][rows].T).astype(ml_dtypes.bfloat16)
        woT = np.ascontiguousarray(Wo[:, rows].T).astype(ml_dtypes.bfloat16)
        qaug = np.zeros((HPC, 3, T), np.float32)
        for h in range(HPC):
            slope = (g * HPC + h + 1) / H
            qaug[h, 0, :] = slope
            qaug[h, 1, :] = slope
            qaug[h, 2, :] = -slope * iota
        m = {
            "xT": xT[b],
            "wqT": wqT,
            "wkT": wkT,
            "wvT": wvT,
            "woT": woT,
            "qaug": qaug.astype(ml_dtypes.bfloat16),
            "kaug": kaug,
            "ident": ident,
            "maskst": maskst,
        }
        if with_bias:
            bv_g = bkv[D : 2 * D][rows]
            m["bvo"] = np.broadcast_to(bv_g[None, :], (P, DG)).copy()
            m["bq2"] = np.ascontiguousarray((bq[rows] * scale).reshape(2, P).T)
            m["bk2"] = np.ascontiguousarray(bkv[0:D][rows].reshape(2, P).T)
        in_maps.append(m)

    res = run_bass_kernel_spmd(nc, in_maps, list(range(N_CORES)), trace=TRACE)
    global LAST_RESULTS
    LAST_RESULTS = res

    out = np.empty((B, T, D), np.float32)
    for b in range(B):
        acc = res.results[b * G]["outT"].astype(np.float32)
        for g in range(1, G):
            acc += res.results[b * G + g]["outT"].astype(np.float32)
        out[b] = acc.T + bo[None, :]
    return out


# revision 18
# speedup vs baseline: 1.0354x; 1.0354x over previous
"""Causal self-attention with ALiBi on 8 trn2 cores.

Sharding: data-parallel over batch (2) x tensor-parallel over head groups (4).
Core c handles batch b = c // 4, head group g = c % 4 (heads 4g..4g+3).

v2 design notes (all timings vs the 261us baseline trace):
- bf16 everywhere on the PE except the x operand (kept f32r, exact):
  weights come from the host as bf16 (halves weight DMA), q/k/v/ex/y are
  written back as bf16 (halves SBUF traffic, enables FWL weight loads).
- ALiBi stays EXACT in bf16 via a 3-row augmented contraction:
  k_aug = [j_hi; j_lo; 1], q_aug = [slope; slope; -slope*i] with
  j = j_hi + j_lo, j_hi multiple of 128 (4-bit mantissa), j_lo < 128
  (7 bits): both products are exact; the -slope*i rounding is constant
  per query column so softmax cancels it exactly.
- ALiBi underflow chunk-skipping: head-slot h (slope >= (h+1)/16 on every
  core) gives exp(score) == 0 in f32 whenever slope*dist > 110, so those
  score/exp/AV chunks are skipped outright (~29% of full chunks).
- HAM warm-up: the PE clock-gates to 1.2 GHz until ~3.4us of sustained
  activity; dummy matmuls on the identity tile warm it during the initial
  x DMA, and the pair-interleaved attention keeps it dense afterwards.
- Attention interleaves the two heads of a pair chunk-by-chunk so the
  exp (ACT engine) latency of one head hides under the other head's
  score/AV matmuls; the output projection of the previous query block is
  emitted between pair0 and pair1 to cover pair0's softmax-normalize.
- Softmax normalize: denominators come out of the AV matmul via a ones
  column in v (row 64 for even heads, 32 for odd); reciprocal_approx_fast
  (single DVE op) + SBUF->SBUF broadcast DMA replace the 3.3us
  InstReciprocal + DRAM round-trip of the baseline.
"""

import sys

sys.path.insert(0, "/opt/trn_rl_repo")

import numpy as np

import concourse.bacc as bacc
import concourse.mybir as mybir
import concourse.tile as tile
from concourse.bass import ds, ts
from concourse.bass_utils import run_bass_kernel_spmd

B, T, D, H, DH = 2, 2048, 1024, 16, 64
G = 4            # head groups (tensor-parallel)
HPC = H // G     # heads per core
DG = D // G      # model dims per core (256)
P = 128
N_CORES = 8
NEG = -1.0e30

F32 = mybir.dt.float32
F32R = mybir.dt.float32r
BF16 = mybir.dt.bfloat16
ADD = mybir.AluOpType.add
MULT = mybir.AluOpType.mult
EXP = mybir.ActivationFunctionType.Exp

TRACE = False
LAST_RESULTS = None

_cache = {}

# ALiBi skip thresholds per head slot: slope >= (slot+1)/16 on every core,
# so dist > 110*16/(slot+1) guarantees exp underflows to 0 in f32.
DIST_MAX = [110 * 16 // (h + 1) for h in range(HPC)]   # [1760, 880, 586, 440]


def _kept_jcs(h, qb):
    """Key chunks contributing to query block qb for head slot h."""
    out = []
    for jc in range(qb * 4 + 4):
        if jc < qb * 4:  # non-diagonal: min distance query->key
            dist = qb * 512 - (jc * P + P - 1)
            if dist > DIST_MAX[h]:
                continue
        out.append(jc)
    return out


def _build(with_bias: bool):
    nc = bacc.Bacc("TRN2", target_bir_lowering=False, debug=False)

    xT_d = nc.dram_tensor("xT", [D, T], F32, kind="ExternalInput").ap()
    wq_d = nc.dram_tensor("wqT", [D, DG], BF16, kind="ExternalInput").ap()
    wk_d = nc.dram_tensor("wkT", [D, DG], BF16, kind="ExternalInput").ap()
    wv_d = nc.dram_tensor("wvT", [D, DG], BF16, kind="ExternalInput").ap()
    wo_d = nc.dram_tensor("woT", [DG, D], BF16, kind="ExternalInput").ap()
    qaug_d = nc.dram_tensor("qaug", [HPC, 3, T], BF16, kind="ExternalInput").ap()
    kaug_d = nc.dram_tensor("kaug", [3, T], BF16, kind="ExternalInput").ap()
    ident_d = nc.dram_tensor("ident", [P, P], BF16, kind="ExternalInput").ap()
    maskst_d = nc.dram_tensor("maskst", [P, P], BF16, kind="ExternalInput").ap()
    if with_bias:
        bvo_d = nc.dram_tensor("bvo", [P, DG], F32, kind="ExternalInput").ap()
        bq_d = nc.dram_tensor("bq2", [P, 2], F32, kind="ExternalInput").ap()
        bk_d = nc.dram_tensor("bk2", [P, 2], F32, kind="ExternalInput").ap()
    out_d = nc.dram_tensor("outT", [D, T], BF16, kind="ExternalOutput").ap()
    dscr_d = nc.dram_tensor("dscratch", [16, 512], F32).ap()

    with tile.TileContext(nc) as tc:
        with (
            tc.tile_pool(name="big", bufs=1) as big,
            tc.tile_pool(name="xtp", bufs=2) as xtp,
            tc.tile_pool(name="stage", bufs=3) as stage,
            tc.tile_pool(name="expp", bufs=3) as expp,
            tc.tile_pool(name="small", bufs=2) as small,
            tc.tile_pool(name="acc", bufs=2, space="PSUM") as accp,
            tc.tile_pool(name="pss", bufs=2, space="PSUM") as pssp,
            tc.tile_pool(name="pyy", bufs=1, space="PSUM") as pyp,
        ):
            # ---- persistent tiles
            wv = [big.tile([P, DG], BF16, tag=f"wv{i}", name=f"wv{i}") for i in range(8)]
            wq = [big.tile([P, DG], BF16, tag=f"wq{i}", name=f"wq{i}") for i in range(8)]
            wk = [big.tile([P, DG], BF16, tag=f"wk{i}", name=f"wk{i}") for i in range(8)]
            wo = [big.tile([P, D], BF16, tag=f"wo{i}", name=f"wo{i}") for i in range(2)]
            qa = [big.tile([67, T], BF16, tag=f"qa{h}", name=f"qa{h}") for h in range(HPC)]
            ka = [big.tile([67, T], BF16, tag=f"ka{h}", name=f"ka{h}") for h in range(HPC)]
            # va layout: per head block of 128 cols; even head: v at +0:64,
            # ones at +64 (AV out rows 0:65); odd head: ones at +32, v at
            # +64:128 (AV out rows 0:128, y at 64:128, denom at 32).
            va = big.tile([P, 16, 4 * P], BF16, tag="va", name="va")
            yt = [big.tile([P, T], BF16, tag=f"yt{m}", name=f"yt{m}") for m in range(2)]
            dn = [big.tile([64, 512], F32, tag=f"dn{m}", name=f"dn{m}") for m in range(2)]
            ident_sb = big.tile([P, P], BF16, tag="ident")
            maskst_sb = big.tile([P, P], BF16, tag="maskst")
            if with_bias:
                bvo = big.tile([P, DG], F32, tag="bvo")
                bq2 = big.tile([P, 2], F32, tag="bq2")
                bk2 = big.tile([P, 2], F32, tag="bk2")

            # ---- loads.  ident first (feeds the HAM warm-up dummies), then
            # the t0 x block in column-quarters so vproj can start early,
            # weights on the scalar queue, remaining x blocks on gpsimd.
            nc.sync.dma_start(out=ident_sb[:], in_=ident_d[:])
            nc.sync.dma_start(out=maskst_sb[:], in_=maskst_d[:])
            for h in range(HPC):
                nc.sync.dma_start(out=qa[h][64:67, :], in_=qaug_d[h])
                nc.sync.dma_start(out=ka[h][64:67, :], in_=kaug_d[:])
            if with_bias:
                nc.sync.dma_start(out=bvo[:], in_=bvo_d[:])
                nc.sync.dma_start(out=bq2[:], in_=bq_d[:])
                nc.sync.dma_start(out=bk2[:], in_=bk_d[:])

            for i in range(8):
                nc.scalar.dma_start(out=wv[i][:], in_=wv_d[ts(i, P), :])

            xtile = [[None] * 8 for _ in range(4)]
            # x loads all go through the gpsimd queue: it is the only one
            # that can cast f32 -> bf16 in flight.  t0 in column-quarters,
            # chunk-major, so vproj chunk ch can start early.
            for i in range(8):
                xtile[0][i] = xtp.tile([P, 512], BF16, tag=f"xt{i}", name=f"x0_{i}")
            for ch in range(4):
                for i in range(8):
                    nc.gpsimd.dma_start(
                        out=xtile[0][i][:, ts(ch, P)],
                        in_=xT_d[ts(i, P), ds(ch * P, P)],
                    )
            for i in range(8):
                nc.scalar.dma_start(out=wq[i][:], in_=wq_d[ts(i, P), :])
            for i in range(8):
                nc.scalar.dma_start(out=wk[i][:], in_=wk_d[ts(i, P), :])
            for tq in range(1, 4):
                for i in range(8):
                    t_ = xtp.tile([P, 512], BF16, tag=f"xt{i}", name=f"x{tq}_{i}")
                    nc.gpsimd.dma_start(out=t_[:], in_=xT_d[ts(i, P), ts(tq, 512)])
                    xtile[tq][i] = t_
            for i in range(2):
                nc.scalar.dma_start(out=wo[i][:], in_=wo_d[ts(i, P), :])

            # va constants: even h ones at h*128+64, odd h junk cols
            # h*128+0:64 zeroed then ones at h*128+32.
            nc.vector.memset(va[:, :, 64:65], 1.0)                    # h0 ones
            nc.vector.memset(va[:, :, 2 * P + 64 : 2 * P + 65], 1.0)  # h2 ones
            nc.vector.memset(va[:, :, P : P + 64], 0.0)               # h1 zeros
            nc.vector.memset(va[:, :, P + 32 : P + 33], 1.0)          # h1 ones
            nc.vector.memset(va[:, :, 3 * P : 3 * P + 64], 0.0)       # h3 zeros
            nc.vector.memset(va[:, :, 3 * P + 32 : 3 * P + 33], 1.0)  # h3 ones
            # denominators scratch: keep finite so reciprocal_approx_fast
            # never sees uninitialized garbage in its unused rows.
            nc.vector.memset(dn[0][:], 1.0)
            nc.vector.memset(dn[1][:], 1.0)

            # ---- HAM warm-up: dummy matmuls while x streams in (junk data,
            # 512-wide moving so each burns ~200-430ns of PE activity).
            junk = big.tile([P, 512], BF16, tag="junk", name="junk")
            nc.vector.memset(junk[:], 0.0)
            for w in range(20):
                pdum = accp.tile([P, 512], F32, tag="acc", name=f"dum{w}")
                nc.tensor.matmul(
                    out=pdum[:], lhsT=ident_sb[:], rhs=junk[:],
                    start=True, stop=True,
                )

            # ---- projections for one 512-wide t block
            def emit_proj(tq):
                xb = xtile[tq]
                # v: [t,dv] chunks; moving = wv (bf16, 256 cols)
                for ch in range(4):
                    tch = tq * 4 + ch
                    pv = accp.tile([P, DG], F32, tag="acc", name=f"pv{tch}")
                    for kc in range(8):
                        nc.tensor.matmul(
                            out=pv[:],
                            lhsT=xb[kc][:, ts(ch, P)],
                            rhs=wv[kc][:],
                            start=(kc == 0),
                            stop=(kc == 7),
                        )
                    # scatter into va: even heads v at cols h*128+0:64,
                    # odd heads v at cols h*128+64:128.  One strided op per
                    # parity via (2, 256)-block views.
                    vdst = va[:, tch, :].rearrange("p (b c) -> p b c", b=2)
                    vsrc = pv[:].rearrange("p (b c) -> p b c", b=2)
                    if with_bias:
                        bsrc = bvo[:].rearrange("p (b c) -> p b c", b=2)
                        nc.vector.tensor_tensor(
                            out=vdst[:, :, 0:64],
                            in0=vsrc[:, :, 0:64],
                            in1=bsrc[:, :, 0:64],
                            op=ADD,
                        )
                        nc.vector.tensor_tensor(
                            out=vdst[:, :, 192:256],
                            in0=vsrc[:, :, 64:128],
                            in1=bsrc[:, :, 64:128],
                            op=ADD,
                        )
                    else:
                        nc.vector.tensor_copy(
                            out=vdst[:, :, 0:64], in_=vsrc[:, :, 0:64]
                        )
                        nc.vector.tensor_copy(
                            out=vdst[:, :, 192:256], in_=vsrc[:, :, 64:128]
                        )
                # q/k: [d',t] via bf16 weight stationaries
                for wt, dst, bias_name in ((wq, qa, "q"), (wk, ka, "k")):
                    bt = (bq2 if bias_name == "q" else bk2) if with_bias else None
                    for mc in range(2):
                        pq = accp.tile([P, 512], F32, tag="acc", name=f"p{bias_name}{tq}_{mc}")
                        for kc in range(8):
                            nc.tensor.matmul(
                                out=pq[:],
                                lhsT=wt[kc][:, ts(mc, P)],
                                rhs=xb[kc][:],
                                start=(kc == 0),
                                stop=(kc == 7),
                            )
                        h_even, h_odd = 2 * mc, 2 * mc + 1
                        if with_bias:
                            nc.vector.tensor_scalar(
                                out=dst[h_even][0:64, ts(tq, 512)],
                                in0=pq[0:64, :],
                                scalar1=bt[0:64, mc : mc + 1],
                                scalar2=None,
                                op0=ADD,
                            )
                            nc.vector.tensor_scalar(
                                out=dst[h_odd][0:64, ts(tq, 512)],
                                in0=pq[64:128, :],
                                scalar1=bt[64:128, mc : mc + 1],
                                scalar2=None,
                                op0=ADD,
                            )
                        else:
                            nc.vector.tensor_copy(
                                out=dst[h_even][0:64, ts(tq, 512)], in_=pq[0:64, :]
                            )
                            nc.vector.tensor_copy(
                                out=dst[h_odd][0:64, ts(tq, 512)], in_=pq[64:128, :]
                            )

            # ---- causal flash attention, pair-interleaved
            def emit_attention_pair(qb, pair):
                o = qb * 512
                hs = (2 * pair, 2 * pair + 1)
                jlists = {h: _kept_jcs(h, qb) for h in hs}
                pys = {}
                for h in hs:
                    py = pyp.tile([P, 512], F32, tag=f"py{h % 2}", name=f"py{qb}_{h}")
                    pys[h] = py
                pend = {h: None for h in hs}
                first_av = {h: True for h in hs}
                all_jcs = sorted(set(jlists[hs[0]]) | set(jlists[hs[1]]))
                for jc in all_jcs:
                    for h in hs:
                        if jc not in jlists[h]:
                            continue
                        r = jc * P - o
                        lo = max(r, 0)
                        ps = pssp.tile(
                            [P, 512], F32, tag=f"ps{h % 2}", name=f"ps{qb}_{h}_{jc}"
                        )
                        if r < 0:
                            nc.tensor.matmul(
                                out=ps[:],
                                lhsT=ka[h][0:67, ts(jc, P)],
                                rhs=qa[h][0:67, ds(o, 512)],
                                start=True,
                                stop=True,
                            )
                        else:
                            nc.tensor.matmul(
                                out=ps[:, lo:512],
                                lhsT=ka[h][0:67, ts(jc, P)],
                                rhs=qa[h][0:67, ds(o + lo, 512 - lo)],
                                start=True,
                                stop=False,
                            )
                            nc.tensor.matmul(
                                out=ps[:, lo : lo + P],
                                lhsT=ident_sb[:],
                                rhs=maskst_sb[:],
                                start=False,
                                stop=True,
                            )
                        ex = expp.tile(
                            [P, 512], BF16, tag=f"ex{h % 2}", name=f"ex{qb}_{h}_{jc}"
                        )
                        nc.scalar.activation(
                            out=ex[:, lo:512], in_=ps[:, lo:512], func=EXP
                        )
                        if pend[h] is not None:
                            pjc, plo, pex = pend[h]
                            _emit_av(
                                nc, pys[h], va, h, pjc, plo, pex, first_av[h], False
                            )
                            first_av[h] = False
                        pend[h] = (jc, lo, ex)
                for h in hs:
                    pjc, plo, pex = pend[h]
                    _emit_av(nc, pys[h], va, h, pjc, plo, pex, first_av[h], True)

                # softmax denominators -> reciprocal -> broadcast -> scale
                h0, h1 = hs
                dnt = dn[pair]
                nc.vector.tensor_copy(out=dnt[0:1, :], in_=pys[h0][64:65, :])
                nc.vector.tensor_copy(out=dnt[32:33, :], in_=pys[h1][32:33, :])
                dn2 = small.tile([64, 512], F32, tag="dn2", name=f"dn2{qb}_{pair}")
                nc.vector.reciprocal_approx_fast(out=dn2[0:33, :], in_=dnt[0:33, :])
                rb = small.tile([P, 512], F32, tag="rb", name=f"rb{qb}_{pair}")
                idx = qb * 2 + pair
                nc.sync.dma_start(out=dscr_d[idx : idx + 1, :], in_=dn2[0:1, :])
                nc.sync.dma_start(
                    out=dscr_d[8 + idx : 9 + idx, :], in_=dn2[32:33, :]
                )
                nc.sync.dma_start(
                    out=rb[0:64, :],
                    in_=dscr_d[idx : idx + 1, :].to_broadcast((64, 512)),
                )
                nc.sync.dma_start(
                    out=rb[64:128, :],
                    in_=dscr_d[8 + idx : 9 + idx, :].to_broadcast((64, 512)),
                )
                nc.vector.tensor_tensor(
                    out=yt[pair][0:64, ds(o, 512)],
                    in0=pys[h0][0:64, :],
                    in1=rb[0:64, :],
                    op=MULT,
                )
                nc.vector.tensor_tensor(
                    out=yt[pair][64:128, ds(o, 512)],
                    in0=pys[h1][64:128, :],
                    in1=rb[64:128, :],
                    op=MULT,
                )

            def emit_outproj(qb):
                for ec in range(8):
                    po = accp.tile([P, 512], F32, tag="acc", name=f"po{qb}_{ec}")
                    for k2 in range(2):
                        nc.tensor.matmul(
                            out=po[:],
                            lhsT=wo[k2][:, ts(ec, P)],
                            rhs=yt[k2][:, ts(qb, 512)],
                            start=(k2 == 0),
                            stop=(k2 == 1),
                        )
                    ob = stage.tile([P, 512], BF16, tag="stage", name="ob")
                    nc.vector.tensor_copy(out=ob[:], in_=po[:])
                    nc.sync.dma_start(out=out_d[ts(ec, P), ts(qb, 512)], in_=ob[:])

            for tq in range(4):
                emit_proj(tq)
                emit_attention_pair(tq, 0)
                if tq > 0:
                    emit_outproj(tq - 1)
                emit_attention_pair(tq, 1)
            emit_outproj(3)

    nc.compile()
    return nc


def _va_cols(h):
    # AV stationary slice for head h within its 128-col va block
    if h % 2 == 0:
        return slice(h * P, h * P + 65)
    return slice(h * P, h * P + P)


def _emit_av(nc, py, va, h, pjc, plo, pex, start, stop):
    cols = _va_cols(h)
    rows = 65 if h % 2 == 0 else P
    nc.tensor.matmul(
        out=py[0:rows, plo:512],
        lhsT=va[:, pjc, cols],
        rhs=pex[:, plo:512],
        start=start,
        stop=stop,
    )


def _get_nc(with_bias: bool):
    if with_bias not in _cache:
        _cache[with_bias] = _build(with_bias)
    return _cache[with_bias]


def kernel(x, freqs_cis, Wq, bq, Wkv, bkv, Wo, bo, **_unused):
    import ml_dtypes

    x = np.asarray(x, np.float32)
    Wq = np.asarray(Wq, np.float32)
    bq = np.asarray(bq, np.float32)
    Wkv = np.asarray(Wkv, np.float32)
    bkv = np.asarray(bkv, np.float32)
    Wo = np.asarray(Wo, np.float32)
    bo = np.asarray(bo, np.float32)

    with_bias = bool(np.any(bq) or np.any(bkv))
    nc = _get_nc(with_bias)

    scale = 1.0 / np.sqrt(DH)
    iota = np.arange(T, dtype=np.float32)

    mm = np.arange(P, dtype=np.float32)
    maskst = np.where(mm[None, :] < mm[:, None], NEG, 0.0).astype(ml_dtypes.bfloat16)
    ident = np.eye(P, dtype=ml_dtypes.bfloat16)

    j_hi = np.floor(iota / P) * P
    j_lo = iota - j_hi
    kaug = np.stack([j_hi, j_lo, np.ones(T, np.float32)]).astype(ml_dtypes.bfloat16)

    xT = [np.ascontiguousarray(x[b].T) for b in range(B)]  # [D, T]

    in_maps = []
    for c in range(N_CORES):
        b, g = divmod(c, G)
        rows = slice(g * DG, (g + 1) * DG)
        wqT = np.ascontiguousarray((Wq[rows] * scale).T).astype(ml_dtypes.bfloat16)
        wkT = np.ascontiguousarray(Wkv[0:D][rows].T).astype(ml_dtypes.bfloat16)
        wvT = np.ascontiguousarray(Wkv[D : 2 * D][rows].T).astype(ml_dtypes.bfloat16)
        woT = np.ascontiguousarray(Wo[:, rows].T).astype(ml_dtypes.bfloat16)
        qaug = np.zeros((HPC, 3, T), np.float32)
        for h in range(HPC):
            slope = (g * HPC + h + 1) / H
            qaug[h, 0, :] = slope
            qaug[h, 1, :] = slope
            qaug[h, 2, :] = -slope * iota
        m = {
            "xT": xT[b],
            "wqT": wqT,
            "wkT": wkT,
            "wvT": wvT,
            "woT": woT,
            "qaug": qaug.astype(ml_dtypes.bfloat16),
            "kaug": kaug,
            "ident": ident,
            "maskst": maskst,
        }
        if with_bias:
            bv_g = bkv[D : 2 * D][rows]
            m["bvo"] = np.broadcast_to(bv_g[None, :], (P, DG)).copy()
            m["bq2"] = np.ascontiguousarray((bq[rows] * scale).reshape(2, P).T)
            m["bk2"] = np.ascontiguousarray(bkv[0:D][rows].reshape(2, P).T)
        in_maps.append(m)

    res = run_bass_kernel_spmd(nc, in_maps, list(range(N_CORES)), trace=TRACE)
    global LAST_RESULTS
    LAST_RESULTS = res

    out = np.empty((B, T, D), np.float32)
    for b in range(B):
        acc = res.results[b * G]["outT"].astype(np.float32)
        for g in range(1, G):
            acc += res.results[b * G + g]["outT"].astype(np.float32)
        out[b] = acc.T + bo[None, :]
    return out


# revision 27
# speedup vs baseline: 1.1711x; 1.1310x over previous
"""Causal self-attention with ALiBi on 8 trn2 cores.

Sharding: data-parallel over batch (2) x tensor-parallel over head groups (4).
Core c handles batch b = c // 4, head group g = c % 4 (heads 4g..4g+3).

v2 design notes (all timings vs the 261us baseline trace):
- bf16 everywhere on the PE except the x operand (kept f32r, exact):
  weights come from the host as bf16 (halves weight DMA), q/k/v/ex/y are
  written back as bf16 (halves SBUF traffic, enables FWL weight loads).
- ALiBi stays EXACT in bf16 via a 3-row augmented contraction:
  k_aug = [j_hi; j_lo; 1], q_aug = [slope; slope; -slope*i] with
  j = j_hi + j_lo, j_hi multiple of 128 (4-bit mantissa), j_lo < 128
  (7 bits): both products are exact; the -slope*i rounding is constant
  per query column so softmax cancels it exactly.
- ALiBi underflow chunk-skipping: head-slot h (slope >= (h+1)/16 on every
  core) gives exp(score) == 0 in f32 whenever slope*dist > 110, so those
  score/exp/AV chunks are skipped outright (~29% of full chunks).
- HAM warm-up: the PE clock-gates to 1.2 GHz until ~3.4us of sustained
  activity; dummy matmuls on the identity tile warm it during the initial
  x DMA, and the pair-interleaved attention keeps it dense afterwards.
- Attention interleaves the two heads of a pair chunk-by-chunk so the
  exp (ACT engine) latency of one head hides under the other head's
  score/AV matmuls; the output projection of the previous query block is
  emitted between pair0 and pair1 to cover pair0's softmax-normalize.
- Softmax normalize: denominators come out of the AV matmul via a ones
  column in v (row 64 for even heads, 32 for odd); reciprocal_approx_fast
  (single DVE op) + SBUF->SBUF broadcast DMA replace the 3.3us
  InstReciprocal + DRAM round-trip of the baseline.
"""

import sys

sys.path.insert(0, "/opt/trn_rl_repo")

import numpy as np

import concourse.bacc as bacc
import concourse.mybir as mybir
import concourse.tile as tile
from concourse.bass import ds, ts
from concourse.bass_utils import run_bass_kernel_spmd

B, T, D, H, DH = 2, 2048, 1024, 16, 64
G = 4            # head groups (tensor-parallel)
HPC = H // G     # heads per core
DG = D // G      # model dims per core (256)
P = 128
N_CORES = 8
NEG = -1.0e30

F32 = mybir.dt.float32
F32R = mybir.dt.float32r
BF16 = mybir.dt.bfloat16
ADD = mybir.AluOpType.add
MULT = mybir.AluOpType.mult
EXP = mybir.ActivationFunctionType.Exp

TRACE = False
LAST_RESULTS = None

_cache = {}

# ALiBi skip thresholds per head slot: slope >= (slot+1)/16 on every core,
# so dist > 110*16/(slot+1) guarantees exp underflows to 0 in f32.
DIST_MAX = [110 * 16 // (h + 1) for h in range(HPC)]   # [1760, 880, 586, 440]


def _kept_jcs(h, qb):
    """Key chunks contributing to query block qb for head slot h."""
    out = []
    for jc in range(qb * 4 + 4):
        if jc < qb * 4:  # non-diagonal: min distance query->key
            dist = qb * 512 - (jc * P + P - 1)
            if dist > DIST_MAX[h]:
                continue
        out.append(jc)
    return out


def _build(with_bias: bool):
    nc = bacc.Bacc("TRN2", target_bir_lowering=False, debug=False)

    xT_d = nc.dram_tensor("xT", [D, T], F32, kind="ExternalInput").ap()
    wq_d = nc.dram_tensor("wqT", [D, DG], BF16, kind="ExternalInput").ap()
    wk_d = nc.dram_tensor("wkT", [D, DG], BF16, kind="ExternalInput").ap()
    wv_d = nc.dram_tensor("wvT", [D, DG], BF16, kind="ExternalInput").ap()
    wo_d = nc.dram_tensor("woT", [DG, D], BF16, kind="ExternalInput").ap()
    qaug_d = nc.dram_tensor("qaug", [HPC, 3, T], BF16, kind="ExternalInput").ap()
    kaug_d = nc.dram_tensor("kaug", [3, T], BF16, kind="ExternalInput").ap()
    ident_d = nc.dram_tensor("ident", [P, P], BF16, kind="ExternalInput").ap()
    maskst_d = nc.dram_tensor("maskst", [P, P], BF16, kind="ExternalInput").ap()
    if with_bias:
        bvo_d = nc.dram_tensor("bvo", [P, DG], F32, kind="ExternalInput").ap()
        bq_d = nc.dram_tensor("bq2", [P, 2], F32, kind="ExternalInput").ap()
        bk_d = nc.dram_tensor("bk2", [P, 2], F32, kind="ExternalInput").ap()
    out_d = nc.dram_tensor("outT", [D, T], BF16, kind="ExternalOutput").ap()
    bcast_d = nc.dram_tensor("bcast33", [33, P], BF16, kind="ExternalInput").ap()

    with tile.TileContext(nc) as tc:
        with (
            tc.tile_pool(name="big", bufs=1) as big,
            tc.tile_pool(name="xtp", bufs=2) as xtp,
            tc.tile_pool(name="stage", bufs=3) as stage,
            tc.tile_pool(name="expp", bufs=3) as expp,
            tc.tile_pool(name="small", bufs=2) as small,
            tc.tile_pool(name="acc", bufs=2, space="PSUM") as accp,
            tc.tile_pool(name="pss", bufs=2, space="PSUM") as pssp,
            tc.tile_pool(name="pyy", bufs=1, space="PSUM") as pyp,
        ):
            # ---- persistent tiles
            wv = [big.tile([P, DG], BF16, tag=f"wv{i}", name=f"wv{i}") for i in range(8)]
            wq = [big.tile([P, DG], BF16, tag=f"wq{i}", name=f"wq{i}") for i in range(8)]
            wk = [big.tile([P, DG], BF16, tag=f"wk{i}", name=f"wk{i}") for i in range(8)]
            wo = [big.tile([P, D], BF16, tag=f"wo{i}", name=f"wo{i}") for i in range(2)]
            qa = [big.tile([67, T], BF16, tag=f"qa{h}", name=f"qa{h}") for h in range(HPC)]
            ka = [big.tile([67, T], BF16, tag=f"ka{h}", name=f"ka{h}") for h in range(HPC)]
            # va layout: per head block of 128 cols; even head: v at +0:64,
            # ones at +64 (AV out rows 0:65); odd head: ones at +32, v at
            # +64:128 (AV out rows 0:128, y at 64:128, denom at 32).
            va = big.tile([P, 16, 4 * P], BF16, tag="va", name="va")
            yt = [big.tile([P, T], BF16, tag=f"yt{m}", name=f"yt{m}") for m in range(2)]
            # per-pair softmax-denominator staging rows (0 and 32 live; the
            # rest must stay finite: the broadcast matmul multiplies them by
            # zero weights, and 0*NaN would poison the result)
            den_sb = [
                big.tile([33, 512], BF16, tag=f"den{m}", name=f"den{m}")
                for m in range(2)
            ]
            bcast_sb = big.tile([33, P], BF16, tag="bcast")
            ident_sb = big.tile([P, P], BF16, tag="ident")
            maskst_sb = big.tile([P, P], BF16, tag="maskst")
            if with_bias:
                bvo = big.tile([P, DG], F32, tag="bvo")
                bq2 = big.tile([P, 2], F32, tag="bq2")
                bk2 = big.tile([P, 2], F32, tag="bk2")

            # ---- loads.  ident first (feeds the HAM warm-up dummies), then
            # the t0 x block in column-quarters so vproj can start early,
            # weights on the scalar queue, remaining x blocks on gpsimd.
            nc.sync.dma_start(out=ident_sb[:], in_=ident_d[:])
            nc.sync.dma_start(out=maskst_sb[:], in_=maskst_d[:])
            nc.sync.dma_start(out=bcast_sb[:], in_=bcast_d[:])
            for h in range(HPC):
                nc.sync.dma_start(out=qa[h][64:67, :], in_=qaug_d[h])
                nc.sync.dma_start(out=ka[h][64:67, :], in_=kaug_d[:])
            if with_bias:
                nc.sync.dma_start(out=bvo[:], in_=bvo_d[:])
                nc.sync.dma_start(out=bq2[:], in_=bq_d[:])
                nc.sync.dma_start(out=bk2[:], in_=bk_d[:])

            for i in range(8):
                nc.scalar.dma_start(out=wv[i][:], in_=wv_d[ts(i, P), :])

            xtile = [[None] * 8 for _ in range(4)]
            # x loads all go through the gpsimd queue: it is the only one
            # that can cast f32 -> bf16 in flight.  t0 in column-quarters,
            # chunk-major, so vproj chunk ch can start early.
            for i in range(8):
                xtile[0][i] = xtp.tile([P, 512], BF16, tag=f"xt{i}", name=f"x0_{i}")
            for ch in range(4):
                for i in range(8):
                    nc.gpsimd.dma_start(
                        out=xtile[0][i][:, ts(ch, P)],
                        in_=xT_d[ts(i, P), ds(ch * P, P)],
                    )
            for i in range(8):
                nc.scalar.dma_start(out=wq[i][:], in_=wq_d[ts(i, P), :])
            for i in range(8):
                nc.scalar.dma_start(out=wk[i][:], in_=wk_d[ts(i, P), :])
            for tq in range(1, 4):
                for i in range(8):
                    t_ = xtp.tile([P, 512], BF16, tag=f"xt{i}", name=f"x{tq}_{i}")
                    nc.gpsimd.dma_start(out=t_[:], in_=xT_d[ts(i, P), ts(tq, 512)])
                    xtile[tq][i] = t_
            for i in range(2):
                nc.scalar.dma_start(out=wo[i][:], in_=wo_d[ts(i, P), :])

            # va constants: even h ones at h*128+64, odd h junk cols
            # h*128+0:64 zeroed then ones at h*128+32.
            nc.vector.memset(va[:, :, 64:65], 1.0)                    # h0 ones
            nc.vector.memset(va[:, :, 2 * P + 64 : 2 * P + 65], 1.0)  # h2 ones
            nc.vector.memset(va[:, :, P : P + 64], 0.0)               # h1 zeros
            nc.vector.memset(va[:, :, P + 32 : P + 33], 1.0)          # h1 ones
            nc.vector.memset(va[:, :, 3 * P : 3 * P + 64], 0.0)       # h3 zeros
            nc.vector.memset(va[:, :, 3 * P + 32 : 3 * P + 33], 1.0)  # h3 ones
            # keep the dead rows of the denominator staging tiles finite
            nc.vector.memset(den_sb[0][:], 1.0)
            nc.vector.memset(den_sb[1][:], 1.0)

            # ---- HAM warm-up: dummy matmuls while x streams in (junk data,
            # 512-wide moving so each burns ~200-430ns of PE activity).
            junk = big.tile([P, 512], BF16, tag="junk", name="junk")
            nc.vector.memset(junk[:], 0.0)
            for w in range(20):
                pdum = accp.tile([P, 512], F32, tag="acc", name=f"dum{w}")
                nc.tensor.matmul(
                    out=pdum[:], lhsT=ident_sb[:], rhs=junk[:],
                    start=True, stop=True,
                )

            # ---- projections for one 512-wide t block
            def emit_proj(tq):
                xb = xtile[tq]
                # v: [t,dv] chunks; moving = wv (bf16, 256 cols)
                for ch in range(4):
                    tch = tq * 4 + ch
                    pv = accp.tile([P, DG], F32, tag="acc", name=f"pv{tch}")
                    for kc in range(8):
                        nc.tensor.matmul(
                            out=pv[:],
                            lhsT=xb[kc][:, ts(ch, P)],
                            rhs=wv[kc][:],
                            start=(kc == 0),
                            stop=(kc == 7),
                        )
                    # scatter into va: even heads v at cols h*128+0:64,
                    # odd heads v at cols h*128+64:128.  One strided op per
                    # parity via (2, 256)-block views.
                    vdst = va[:, tch, :].rearrange("p (b c) -> p b c", b=2)
                    vsrc = pv[:].rearrange("p (b c) -> p b c", b=2)
                    if with_bias:
                        bsrc = bvo[:].rearrange("p (b c) -> p b c", b=2)
                        nc.vector.tensor_tensor(
                            out=vdst[:, :, 0:64],
                            in0=vsrc[:, :, 0:64],
                            in1=bsrc[:, :, 0:64],
                            op=ADD,
                        )
                        nc.vector.tensor_tensor(
                            out=vdst[:, :, 192:256],
                            in0=vsrc[:, :, 64:128],
                            in1=bsrc[:, :, 64:128],
                            op=ADD,
                        )
                    else:
                        nc.vector.tensor_copy(
                            out=vdst[:, :, 0:64], in_=vsrc[:, :, 0:64]
                        )
                        nc.vector.tensor_copy(
                            out=vdst[:, :, 192:256], in_=vsrc[:, :, 64:128]
                        )
                # q/k: [d',t] via bf16 weight stationaries
                for wt, dst, bias_name in ((wq, qa, "q"), (wk, ka, "k")):
                    bt = (bq2 if bias_name == "q" else bk2) if with_bias else None
                    for mc in range(2):
                        pq = accp.tile([P, 512], F32, tag="acc", name=f"p{bias_name}{tq}_{mc}")
                        for kc in range(8):
                            nc.tensor.matmul(
                                out=pq[:],
                                lhsT=wt[kc][:, ts(mc, P)],
                                rhs=xb[kc][:],
                                start=(kc == 0),
                                stop=(kc == 7),
                            )
                        h_even, h_odd = 2 * mc, 2 * mc + 1
                        if with_bias:
                            nc.vector.tensor_scalar(
                                out=dst[h_even][0:64, ts(tq, 512)],
                                in0=pq[0:64, :],
                                scalar1=bt[0:64, mc : mc + 1],
                                scalar2=None,
                                op0=ADD,
                            )
                            nc.vector.tensor_scalar(
                                out=dst[h_odd][0:64, ts(tq, 512)],
                                in0=pq[64:128, :],
                                scalar1=bt[64:128, mc : mc + 1],
                                scalar2=None,
                                op0=ADD,
                            )
                        else:
                            nc.vector.tensor_copy(
                                out=dst[h_even][0:64, ts(tq, 512)], in_=pq[0:64, :]
                            )
                            nc.vector.tensor_copy(
                                out=dst[h_odd][0:64, ts(tq, 512)], in_=pq[64:128, :]
                            )

            # ---- causal flash attention, pair-interleaved.  Both heads of
            # the pair write score chunks into one double-wide PSUM tile so
            # a single exp covers both (halves the ACT per-instr overhead).
            def emit_attention_pair(qb, pair):
                o = qb * 512
                hs = (2 * pair, 2 * pair + 1)
                jlists = {h: _kept_jcs(h, qb) for h in hs}
                pys = {}
                for h in hs:
                    py = pyp.tile([P, 512], F32, tag=f"py{h % 2}", name=f"py{qb}_{h}")
                    pys[h] = py
                pend = {h: None for h in hs}
                first_av = {h: True for h in hs}
                all_jcs = sorted(set(jlists[hs[0]]) | set(jlists[hs[1]]))
                for jc in all_jcs:
                    present = [h for h in hs if jc in jlists[h]]
                    r = jc * P - o
                    lo = max(r, 0)
                    ps = pssp.tile(
                        [P, 2, 512], F32, tag="ps", name=f"ps{qb}_{pair}_{jc}"
                    )
                    for h in present:
                        hx = h % 2
                        if r < 0:
                            nc.tensor.matmul(
                                out=ps[:, hx, :],
                                lhsT=ka[h][0:67, ts(jc, P)],
                                rhs=qa[h][0:67, ds(o, 512)],
                                start=True,
                                stop=True,
                            )
                        else:
                            nc.tensor.matmul(
                                out=ps[:, hx, lo:512],
                                lhsT=ka[h][0:67, ts(jc, P)],
                                rhs=qa[h][0:67, ds(o + lo, 512 - lo)],
                                start=True,
                                stop=False,
                            )
                            nc.tensor.matmul(
                                out=ps[:, hx, lo : lo + P],
                                lhsT=ident_sb[:],
                                rhs=maskst_sb[:],
                                start=False,
                                stop=True,
                            )
                    ex = expp.tile(
                        [P, 2, 512], BF16, tag="ex", name=f"ex{qb}_{pair}_{jc}"
                    )
                    if len(present) == 2:
                        nc.scalar.activation(
                            out=ex[:, :, lo:512], in_=ps[:, :, lo:512], func=EXP
                        )
                    else:
                        hx = present[0] % 2
                        nc.scalar.activation(
                            out=ex[:, hx, lo:512], in_=ps[:, hx, lo:512], func=EXP
                        )
                    for h in present:
                        if pend[h] is not None:
                            pjc, plo, pex = pend[h]
                            _emit_av(
                                nc, pys[h], va, h, pjc, plo, pex, first_av[h], False
                            )
                            first_av[h] = False
                        pend[h] = (jc, lo, ex)
                for h in hs:
                    pjc, plo, pex = pend[h]
                    _emit_av(nc, pys[h], va, h, pjc, plo, pex, first_av[h], True)

                # softmax normalize: stage raw denominators (rows 0 / 32),
                # broadcast via a ones-matmul on the PE, one big reciprocal,
                # then per-head column scaling.
                h0, h1 = hs
                dst = den_sb[pair]
                nc.vector.tensor_copy(out=dst[0:1, :], in_=pys[h0][64:65, :])
                nc.scalar.copy(out=dst[32:33, :], in_=pys[h1][32:33, :])
                rb_ps = pssp.tile([P, 512], F32, tag="ps", name=f"rbp{qb}_{pair}")
                nc.tensor.matmul(
                    out=rb_ps[:], lhsT=bcast_sb[:], rhs=dst[:], start=True, stop=True
                )
                rb = small.tile([P, 512], F32, tag="rb", name=f"rb{qb}_{pair}")
                nc.vector.reciprocal_approx_fast(out=rb[:], in_=rb_ps[:])
                nc.vector.tensor_tensor(
                    out=yt[pair][0:64, ds(o, 512)],
                    in0=pys[h0][0:64, :],
                    in1=rb[0:64, :],
                    op=MULT,
                )
                nc.vector.tensor_tensor(
                    out=yt[pair][64:128, ds(o, 512)],
                    in0=pys[h1][64:128, :],
                    in1=rb[64:128, :],
                    op=MULT,
                )

            def emit_outproj(qb):
                for ec in range(8):
                    po = accp.tile([P, 512], F32, tag="acc", name=f"po{qb}_{ec}")
                    for k2 in range(2):
                        nc.tensor.matmul(
                            out=po[:],
                            lhsT=wo[k2][:, ts(ec, P)],
                            rhs=yt[k2][:, ts(qb, 512)],
                            start=(k2 == 0),
                            stop=(k2 == 1),
                        )
                    ob = stage.tile([P, 512], BF16, tag="stage", name="ob")
                    if ec % 2 == 0:
                        nc.vector.tensor_copy(out=ob[:], in_=po[:])
                        nc.sync.dma_start(out=out_d[ts(ec, P), ts(qb, 512)], in_=ob[:])
                    else:
                        nc.scalar.copy(out=ob[:], in_=po[:])
                        nc.scalar.dma_start(
                            out=out_d[ts(ec, P), ts(qb, 512)], in_=ob[:]
                        )

            for tq in range(4):
                emit_proj(tq)
                emit_attention_pair(tq, 0)
                if tq > 0:
                    emit_outproj(tq - 1)
                emit_attention_pair(tq, 1)
            emit_outproj(3)

    nc.compile()
    return nc


def _va_cols(h):
    # AV stationary slice for head h within its 128-col va block
    if h % 2 == 0:
        return slice(h * P, h * P + 65)
    return slice(h * P, h * P + P)


def _emit_av(nc, py, va, h, pjc, plo, pex, start, stop):
    cols = _va_cols(h)
    rows = 65 if h % 2 == 0 else P
    nc.tensor.matmul(
        out=py[0:rows, plo:512],
        lhsT=va[:, pjc, cols],
        rhs=pex[:, h % 2, plo:512],
        start=start,
        stop=stop,
    )


def _get_nc(with_bias: bool):
    if with_bias not in _cache:
        _cache[with_bias] = _build(with_bias)
    return _cache[with_bias]


def kernel(x, freqs_cis, Wq, bq, Wkv, bkv, Wo, bo, **_unused):
    import ml_dtypes

    x = np.asarray(x, np.float32)
    Wq = np.asarray(Wq, np.float32)
    bq = np.asarray(bq, np.float32)
    Wkv = np.asarray(Wkv, np.float32)
    bkv = np.asarray(bkv, np.float32)
    Wo = np.asarray(Wo, np.float32)
    bo = np.asarray(bo, np.float32)

    with_bias = bool(np.any(bq) or np.any(bkv))
    nc = _get_nc(with_bias)

    scale = 1.0 / np.sqrt(DH)
    iota = np.arange(T, dtype=np.float32)

    mm = np.arange(P, dtype=np.float32)
    maskst = np.where(mm[None, :] < mm[:, None], NEG, 0.0).astype(ml_dtypes.bfloat16)
    ident = np.eye(P, dtype=ml_dtypes.bfloat16)

    j_hi = np.floor(iota / P) * P
    j_lo = iota - j_hi
    kaug = np.stack([j_hi, j_lo, np.ones(T, np.float32)]).astype(ml_dtypes.bfloat16)

    bcast33 = np.zeros((33, P), np.float32)
    bcast33[0, 0:64] = 1.0
    bcast33[32, 64:128] = 1.0
    bcast33 = bcast33.astype(ml_dtypes.bfloat16)

    xT = [np.ascontiguousarray(x[b].T) for b in range(B)]  # [D, T]

    in_maps = []
    for c in range(N_CORES):
        b, g = divmod(c, G)
        rows = slice(g * DG, (g + 1) * DG)
        wqT = np.ascontiguousarray((Wq[rows] * scale).T).astype(ml_dtypes.bfloat16)
        wkT = np.ascontiguousarray(Wkv[0:D][rows].T).astype(ml_dtypes.bfloat16)
        wvT = np.ascontiguousarray(Wkv[D : 2 * D][rows].T).astype(ml_dtypes.bfloat16)
        woT = np.ascontiguousarray(Wo[:, rows].T).astype(ml_dtypes.bfloat16)
        qaug = np.zeros((HPC, 3, T), np.float32)
        for h in range(HPC):
            slope = (g * HPC + h + 1) / H
            qaug[h, 0, :] = slope
            qaug[h, 1, :] = slope
            qaug[h, 2, :] = -slope * iota
        m = {
            "xT": xT[b],
            "wqT": wqT,
            "wkT": wkT,
            "wvT": wvT,
            "woT": woT,
            "qaug": qaug.astype(ml_dtypes.bfloat16),
            "kaug": kaug,
            "ident": ident,
            "maskst": maskst,
            "bcast33": bcast33,
        }
        if with_bias:
            bv_g = bkv[D : 2 * D][rows]
            m["bvo"] = np.broadcast_to(bv_g[None, :], (P, DG)).copy()
            m["bq2"] = np.ascontiguousarray((bq[rows] * scale).reshape(2, P).T)
            m["bk2"] = np.ascontiguousarray(bkv[0:D][rows].reshape(2, P).T)
        in_maps.append(m)

    res = run_bass_kernel_spmd(nc, in_maps, list(range(N_CORES)), trace=TRACE)
    global LAST_RESULTS
    LAST_RESULTS = res

    out = np.empty((B, T, D), np.float32)
    for b in range(B):
        acc = res.results[b * G]["outT"].astype(np.float32)
        for g in range(1, G):
            acc += res.results[b * G + g]["outT"].astype(np.float32)
        out[b] = acc.T + bo[None, :]
    return out


# revision 30
# speedup vs baseline: 1.2185x; 1.0404x over previous
"""Causal self-attention with ALiBi on 8 trn2 cores.

Sharding: data-parallel over batch (2) x tensor-parallel over head groups (4).
Core c handles batch b = c // 4, head group g = c % 4 (heads 4g..4g+3).

v2 design notes (all timings vs the 261us baseline trace):
- bf16 everywhere on the PE except the x operand (kept f32r, exact):
  weights come from the host as bf16 (halves weight DMA), q/k/v/ex/y are
  written back as bf16 (halves SBUF traffic, enables FWL weight loads).
- ALiBi stays EXACT in bf16 via a 3-row augmented contraction:
  k_aug = [j_hi; j_lo; 1], q_aug = [slope; slope; -slope*i] with
  j = j_hi + j_lo, j_hi multiple of 128 (4-bit mantissa), j_lo < 128
  (7 bits): both products are exact; the -slope*i rounding is constant
  per query column so softmax cancels it exactly.
- ALiBi underflow chunk-skipping: head-slot h (slope >= (h+1)/16 on every
  core) gives exp(score) == 0 in f32 whenever slope*dist > 110, so those
  score/exp/AV chunks are skipped outright (~29% of full chunks).
- HAM warm-up: the PE clock-gates to 1.2 GHz until ~3.4us of sustained
  activity; dummy matmuls on the identity tile warm it during the initial
  x DMA, and the pair-interleaved attention keeps it dense afterwards.
- Attention interleaves the two heads of a pair chunk-by-chunk so the
  exp (ACT engine) latency of one head hides under the other head's
  score/AV matmuls; the output projection of the previous query block is
  emitted between pair0 and pair1 to cover pair0's softmax-normalize.
- Softmax normalize: denominators come out of the AV matmul via a ones
  column in v (row 64 for even heads, 32 for odd); reciprocal_approx_fast
  (single DVE op) + SBUF->SBUF broadcast DMA replace the 3.3us
  InstReciprocal + DRAM round-trip of the baseline.
"""

import sys

sys.path.insert(0, "/opt/trn_rl_repo")

import numpy as np

import concourse.bacc as bacc
import concourse.mybir as mybir
import concourse.tile as tile
from concourse.bass import ds, ts
from concourse.bass_utils import run_bass_kernel_spmd

B, T, D, H, DH = 2, 2048, 1024, 16, 64
G = 4            # head groups (tensor-parallel)
HPC = H // G     # heads per core
DG = D // G      # model dims per core (256)
P = 128
N_CORES = 8
NEG = -1.0e30

F32 = mybir.dt.float32
F32R = mybir.dt.float32r
BF16 = mybir.dt.bfloat16
ADD = mybir.AluOpType.add
MULT = mybir.AluOpType.mult
EXP = mybir.ActivationFunctionType.Exp

TRACE = False
LAST_RESULTS = None

_cache = {}

# ALiBi skip thresholds per head slot: slope >= (slot+1)/16 on every core,
# so dist > 110*16/(slot+1) guarantees exp underflows to 0 in f32.
DIST_MAX = [110 * 16 // (h + 1) for h in range(HPC)]   # [1760, 880, 586, 440]


def _kept_jcs(h, qb):
    """Key chunks contributing to query block qb for head slot h."""
    out = []
    for jc in range(qb * 4 + 4):
        if jc < qb * 4:  # non-diagonal: min distance query->key
            dist = qb * 512 - (jc * P + P - 1)
            if dist > DIST_MAX[h]:
                continue
        out.append(jc)
    return out


def _build(with_bias: bool):
    nc = bacc.Bacc("TRN2", target_bir_lowering=False, debug=False)

    xT_d = nc.dram_tensor("xT", [D, T], F32, kind="ExternalInput").ap()
    wq_d = nc.dram_tensor("wqT", [D, DG], BF16, kind="ExternalInput").ap()
    wk_d = nc.dram_tensor("wkT", [D, DG], BF16, kind="ExternalInput").ap()
    wv_d = nc.dram_tensor("wvT", [D, DG], BF16, kind="ExternalInput").ap()
    wo_d = nc.dram_tensor("woT", [DG, D], BF16, kind="ExternalInput").ap()
    qaug_d = nc.dram_tensor("qaug", [HPC, 3, T], BF16, kind="ExternalInput").ap()
    kaug_d = nc.dram_tensor("kaug", [3, T], BF16, kind="ExternalInput").ap()
    ident_d = nc.dram_tensor("ident", [P, P], BF16, kind="ExternalInput").ap()
    maskst_d = nc.dram_tensor("maskst", [P, P], BF16, kind="ExternalInput").ap()
    if with_bias:
        bvo_d = nc.dram_tensor("bvo", [P, DG], F32, kind="ExternalInput").ap()
        bq_d = nc.dram_tensor("bq2", [P, 2], F32, kind="ExternalInput").ap()
        bk_d = nc.dram_tensor("bk2", [P, 2], F32, kind="ExternalInput").ap()
    out_d = nc.dram_tensor("outT", [D, T], BF16, kind="ExternalOutput").ap()
    bcast_d = nc.dram_tensor("bcast33", [33, P], BF16, kind="ExternalInput").ap()

    with tile.TileContext(nc) as tc:
        with (
            tc.tile_pool(name="big", bufs=1) as big,
            tc.tile_pool(name="xtp", bufs=2) as xtp,
            tc.tile_pool(name="stage", bufs=3) as stage,
            tc.tile_pool(name="expp", bufs=3) as expp,
            tc.tile_pool(name="small", bufs=2) as small,
            tc.tile_pool(name="acc", bufs=2, space="PSUM") as accp,
            tc.tile_pool(name="pss", bufs=2, space="PSUM") as pssp,
            tc.tile_pool(name="pyy", bufs=1, space="PSUM") as pyp,
        ):
            # ---- persistent tiles
            wv = [big.tile([P, DG], BF16, tag=f"wv{i}", name=f"wv{i}") for i in range(8)]
            wq = [big.tile([P, DG], BF16, tag=f"wq{i}", name=f"wq{i}") for i in range(8)]
            wk = [big.tile([P, DG], BF16, tag=f"wk{i}", name=f"wk{i}") for i in range(8)]
            wo = [big.tile([P, D], BF16, tag=f"wo{i}", name=f"wo{i}") for i in range(2)]
            qa = [big.tile([67, T], BF16, tag=f"qa{h}", name=f"qa{h}") for h in range(HPC)]
            ka = [big.tile([67, T], BF16, tag=f"ka{h}", name=f"ka{h}") for h in range(HPC)]
            # va layout: per head block of 128 cols; even head: v at +0:64,
            # ones at +64 (AV out rows 0:65); odd head: ones at +32, v at
            # +64:128 (AV out rows 0:128, y at 64:128, denom at 32).
            va = big.tile([P, 16, 4 * P], BF16, tag="va", name="va")
            yt = [big.tile([P, T], BF16, tag=f"yt{m}", name=f"yt{m}") for m in range(2)]
            # per-pair softmax-denominator staging rows (0 and 32 live; the
            # rest must stay finite: the broadcast matmul multiplies them by
            # zero weights, and 0*NaN would poison the result)
            den_sb = [
                big.tile([33, 512], BF16, tag=f"den{m}", name=f"den{m}")
                for m in range(2)
            ]
            bcast_sb = big.tile([33, P], BF16, tag="bcast")
            ident_sb = big.tile([P, P], BF16, tag="ident")
            maskst_sb = big.tile([P, P], BF16, tag="maskst")
            if with_bias:
                bvo = big.tile([P, DG], F32, tag="bvo")
                bq2 = big.tile([P, 2], F32, tag="bq2")
                bk2 = big.tile([P, 2], F32, tag="bk2")

            # ---- loads.  ident first (feeds the HAM warm-up dummies), then
            # the t0 x block in column-quarters so vproj can start early,
            # weights on the scalar queue, remaining x blocks on gpsimd.
            nc.sync.dma_start(out=ident_sb[:], in_=ident_d[:])
            nc.sync.dma_start(out=maskst_sb[:], in_=maskst_d[:])
            nc.sync.dma_start(out=bcast_sb[:], in_=bcast_d[:])
            for h in range(HPC):
                nc.sync.dma_start(out=qa[h][64:67, :], in_=qaug_d[h])
                nc.sync.dma_start(out=ka[h][64:67, :], in_=kaug_d[:])
            if with_bias:
                nc.sync.dma_start(out=bvo[:], in_=bvo_d[:])
                nc.sync.dma_start(out=bq2[:], in_=bq_d[:])
                nc.sync.dma_start(out=bk2[:], in_=bk_d[:])

            for i in range(8):
                nc.scalar.dma_start(out=wv[i][:], in_=wv_d[ts(i, P), :])

            xtile = [[None] * 8 for _ in range(4)]
            # x loads all go through the gpsimd queue: it is the only one
            # that can cast f32 -> bf16 in flight.  Whole tiles: software-DGE
            # descriptor generation costs ~0.8us per dma_start, so fewer,
            # larger transfers win.
            for i in range(8):
                xtile[0][i] = xtp.tile([P, 512], BF16, tag=f"xt{i}", name=f"x0_{i}")
                nc.gpsimd.dma_start(out=xtile[0][i][:], in_=xT_d[ts(i, P), 0:512])
            for i in range(8):
                nc.scalar.dma_start(out=wq[i][:], in_=wq_d[ts(i, P), :])
            for i in range(8):
                nc.scalar.dma_start(out=wk[i][:], in_=wk_d[ts(i, P), :])
            for tq in range(1, 4):
                for i in range(8):
                    t_ = xtp.tile([P, 512], BF16, tag=f"xt{i}", name=f"x{tq}_{i}")
                    nc.gpsimd.dma_start(out=t_[:], in_=xT_d[ts(i, P), ts(tq, 512)])
                    xtile[tq][i] = t_
            for i in range(2):
                nc.scalar.dma_start(out=wo[i][:], in_=wo_d[ts(i, P), :])

            # va constants: even h ones at h*128+64, odd h junk cols
            # h*128+0:64 zeroed then ones at h*128+32.
            nc.vector.memset(va[:, :, 64:65], 1.0)                    # h0 ones
            nc.vector.memset(va[:, :, 2 * P + 64 : 2 * P + 65], 1.0)  # h2 ones
            nc.vector.memset(va[:, :, P : P + 64], 0.0)               # h1 zeros
            nc.vector.memset(va[:, :, P + 32 : P + 33], 1.0)          # h1 ones
            nc.vector.memset(va[:, :, 3 * P : 3 * P + 64], 0.0)       # h3 zeros
            nc.vector.memset(va[:, :, 3 * P + 32 : 3 * P + 33], 1.0)  # h3 ones
            # keep the dead rows of the denominator staging tiles finite
            nc.vector.memset(den_sb[0][:], 1.0)
            nc.vector.memset(den_sb[1][:], 1.0)

            # ---- HAM warm-up: dummy matmuls while x streams in (junk data,
            # 512-wide moving so each burns ~200-430ns of PE activity).
            junk = big.tile([P, 512], BF16, tag="junk", name="junk")
            nc.vector.memset(junk[:], 0.0)

            def emit_dummies(n, label):
                for w in range(n):
                    pdum = accp.tile([P, 512], F32, tag="acc", name=f"dum{label}_{w}")
                    nc.tensor.matmul(
                        out=pdum[:], lhsT=ident_sb[:], rhs=junk[:],
                        start=True, stop=True,
                    )

            emit_dummies(40, "pre")

            # ---- projections for one 512-wide t block
            def emit_proj(tq):
                xb = xtile[tq]
                # v: [t,dv] chunks; moving = wv (bf16, 256 cols)
                for ch in range(4):
                    tch = tq * 4 + ch
                    pv = accp.tile([P, DG], F32, tag="acc", name=f"pv{tch}")
                    for kc in range(8):
                        nc.tensor.matmul(
                            out=pv[:],
                            lhsT=xb[kc][:, ts(ch, P)],
                            rhs=wv[kc][:],
                            start=(kc == 0),
                            stop=(kc == 7),
                        )
                    # scatter into va: even heads v at cols h*128+0:64,
                    # odd heads v at cols h*128+64:128.  One strided op per
                    # parity via (2, 256)-block views.
                    vdst = va[:, tch, :].rearrange("p (b c) -> p b c", b=2)
                    vsrc = pv[:].rearrange("p (b c) -> p b c", b=2)
                    if with_bias:
                        bsrc = bvo[:].rearrange("p (b c) -> p b c", b=2)
                        nc.vector.tensor_tensor(
                            out=vdst[:, :, 0:64],
                            in0=vsrc[:, :, 0:64],
                            in1=bsrc[:, :, 0:64],
                            op=ADD,
                        )
                        nc.vector.tensor_tensor(
                            out=vdst[:, :, 192:256],
                            in0=vsrc[:, :, 64:128],
                            in1=bsrc[:, :, 64:128],
                            op=ADD,
                        )
                    else:
                        nc.vector.tensor_copy(
                            out=vdst[:, :, 0:64], in_=vsrc[:, :, 0:64]
                        )
                        nc.vector.tensor_copy(
                            out=vdst[:, :, 192:256], in_=vsrc[:, :, 64:128]
                        )
                # q/k: [d',t] via bf16 weight stationaries
                for wt, dst, bias_name in ((wq, qa, "q"), (wk, ka, "k")):
                    bt = (bq2 if bias_name == "q" else bk2) if with_bias else None
                    for mc in range(2):
                        pq = accp.tile([P, 512], F32, tag="acc", name=f"p{bias_name}{tq}_{mc}")
                        for kc in range(8):
                            nc.tensor.matmul(
                                out=pq[:],
                                lhsT=wt[kc][:, ts(mc, P)],
                                rhs=xb[kc][:],
                                start=(kc == 0),
                                stop=(kc == 7),
                            )
                        h_even, h_odd = 2 * mc, 2 * mc + 1
                        if with_bias:
                            nc.vector.tensor_scalar(
                                out=dst[h_even][0:64, ts(tq, 512)],
                                in0=pq[0:64, :],
                                scalar1=bt[0:64, mc : mc + 1],
                                scalar2=None,
                                op0=ADD,
                            )
                            nc.vector.tensor_scalar(
                                out=dst[h_odd][0:64, ts(tq, 512)],
                                in0=pq[64:128, :],
                                scalar1=bt[64:128, mc : mc + 1],
                                scalar2=None,
                                op0=ADD,
                            )
                        else:
                            nc.vector.tensor_copy(
                                out=dst[h_even][0:64, ts(tq, 512)], in_=pq[0:64, :]
                            )
                            nc.vector.tensor_copy(
                                out=dst[h_odd][0:64, ts(tq, 512)], in_=pq[64:128, :]
                            )

            # ---- causal flash attention, pair-interleaved.  Both heads of
            # the pair write score chunks into one double-wide PSUM tile so
            # a single exp covers both (halves the ACT per-instr overhead).
            def emit_attention_pair(qb, pair):
                o = qb * 512
                hs = (2 * pair, 2 * pair + 1)
                jlists = {h: _kept_jcs(h, qb) for h in hs}
                pys = {}
                for h in hs:
                    py = pyp.tile([P, 512], F32, tag=f"py{h % 2}", name=f"py{qb}_{h}")
                    pys[h] = py
                pend = {h: None for h in hs}
                first_av = {h: True for h in hs}
                all_jcs = sorted(set(jlists[hs[0]]) | set(jlists[hs[1]]))
                for jc in all_jcs:
                    present = [h for h in hs if jc in jlists[h]]
                    r = jc * P - o
                    lo = max(r, 0)
                    ps = pssp.tile(
                        [P, 2, 512], F32, tag="ps", name=f"ps{qb}_{pair}_{jc}"
                    )
                    for h in present:
                        hx = h % 2
                        if r < 0:
                            nc.tensor.matmul(
                                out=ps[:, hx, :],
                                lhsT=ka[h][0:67, ts(jc, P)],
                                rhs=qa[h][0:67, ds(o, 512)],
                                start=True,
                                stop=True,
                            )
                        else:
                            nc.tensor.matmul(
                                out=ps[:, hx, lo:512],
                                lhsT=ka[h][0:67, ts(jc, P)],
                                rhs=qa[h][0:67, ds(o + lo, 512 - lo)],
                                start=True,
                                stop=False,
                            )
                            nc.tensor.matmul(
                                out=ps[:, hx, lo : lo + P],
                                lhsT=ident_sb[:],
                                rhs=maskst_sb[:],
                                start=False,
                                stop=True,
                            )
                    ex = expp.tile(
                        [P, 2, 512], BF16, tag="ex", name=f"ex{qb}_{pair}_{jc}"
                    )
                    if len(present) == 2:
                        nc.scalar.activation(
                            out=ex[:, :, lo:512], in_=ps[:, :, lo:512], func=EXP
                        )
                    else:
                        hx = present[0] % 2
                        nc.scalar.activation(
                            out=ex[:, hx, lo:512], in_=ps[:, hx, lo:512], func=EXP
                        )
                    for h in present:
                        if pend[h] is not None:
                            pjc, plo, pex = pend[h]
                            _emit_av(
                                nc, pys[h], va, h, pjc, plo, pex, first_av[h], False
                            )
                            first_av[h] = False
                        pend[h] = (jc, lo, ex)
                for h in hs:
                    pjc, plo, pex = pend[h]
                    _emit_av(nc, pys[h], va, h, pjc, plo, pex, first_av[h], True)

                # softmax normalize: stage raw denominators (rows 0 / 32),
                # broadcast via a ones-matmul on the PE, one big reciprocal,
                # then per-head column scaling.
                h0, h1 = hs
                dst = den_sb[pair]
                nc.vector.tensor_copy(out=dst[0:1, :], in_=pys[h0][64:65, :])
                nc.scalar.copy(out=dst[32:33, :], in_=pys[h1][32:33, :])
                rb_ps = pssp.tile([P, 512], F32, tag="ps", name=f"rbp{qb}_{pair}")
                nc.tensor.matmul(
                    out=rb_ps[:], lhsT=bcast_sb[:], rhs=dst[:], start=True, stop=True
                )
                rb = small.tile([P, 512], F32, tag="rb", name=f"rb{qb}_{pair}")
                nc.vector.reciprocal_approx_fast(out=rb[:], in_=rb_ps[:])
                nc.vector.tensor_tensor(
                    out=yt[pair][0:64, ds(o, 512)],
                    in0=pys[h0][0:64, :],
                    in1=rb[0:64, :],
                    op=MULT,
                )
                nc.vector.tensor_tensor(
                    out=yt[pair][64:128, ds(o, 512)],
                    in0=pys[h1][64:128, :],
                    in1=rb[64:128, :],
                    op=MULT,
                )

            def emit_outproj(qb):
                for ec in range(8):
                    po = accp.tile([P, 512], F32, tag="acc", name=f"po{qb}_{ec}")
                    for k2 in range(2):
                        nc.tensor.matmul(
                            out=po[:],
                            lhsT=wo[k2][:, ts(ec, P)],
                            rhs=yt[k2][:, ts(qb, 512)],
                            start=(k2 == 0),
                            stop=(k2 == 1),
                        )
                    ob = stage.tile([P, 512], BF16, tag="stage", name="ob")
                    if ec % 2 == 0:
                        nc.vector.tensor_copy(out=ob[:], in_=po[:])
                        nc.sync.dma_start(out=out_d[ts(ec, P), ts(qb, 512)], in_=ob[:])
                    else:
                        nc.scalar.copy(out=ob[:], in_=po[:])
                        nc.scalar.dma_start(
                            out=out_d[ts(ec, P), ts(qb, 512)], in_=ob[:]
                        )

            for tq in range(4):
                emit_proj(tq)
                emit_attention_pair(tq, 0)
                if tq > 0:
                    emit_outproj(tq - 1)
                emit_attention_pair(tq, 1)
            # keep HAM warm while the last pair's normalize chain drains
            emit_dummies(6, "tail")
            emit_outproj(3)

    nc.compile()
    return nc


def _va_cols(h):
    # AV stationary slice for head h within its 128-col va block
    if h % 2 == 0:
        return slice(h * P, h * P + 65)
    return slice(h * P, h * P + P)


def _emit_av(nc, py, va, h, pjc, plo, pex, start, stop):
    cols = _va_cols(h)
    rows = 65 if h % 2 == 0 else P
    nc.tensor.matmul(
        out=py[0:rows, plo:512],
        lhsT=va[:, pjc, cols],
        rhs=pex[:, h % 2, plo:512],
        start=start,
        stop=stop,
    )


def _get_nc(with_bias: bool):
    if with_bias not in _cache:
        _cache[with_bias] = _build(with_bias)
    return _cache[with_bias]


def kernel(x, freqs_cis, Wq, bq, Wkv, bkv, Wo, bo, **_unused):
    import ml_dtypes

    x = np.asarray(x, np.float32)
    Wq = np.asarray(Wq, np.float32)
    bq = np.asarray(bq, np.float32)
    Wkv = np.asarray(Wkv, np.float32)
    bkv = np.asarray(bkv, np.float32)
    Wo = np.asarray(Wo, np.float32)
    bo = np.asarray(bo, np.float32)

    with_bias = bool(np.any(bq) or np.any(bkv))
    nc = _get_nc(with_bias)

    scale = 1.0 / np.sqrt(DH)
    iota = np.arange(T, dtype=np.float32)

    mm = np.arange(P, dtype=np.float32)
    maskst = np.where(mm[None, :] < mm[:, None], NEG, 0.0).astype(ml_dtypes.bfloat16)
    ident = np.eye(P, dtype=ml_dtypes.bfloat16)

    j_hi = np.floor(iota / P) * P
    j_lo = iota - j_hi
    kaug = np.stack([j_hi, j_lo, np.ones(T, np.float32)]).astype(ml_dtypes.bfloat16)

    bcast33 = np.zeros((33, P), np.float32)
    bcast33[0, 0:64] = 1.0
    bcast33[32, 64:128] = 1.0
    bcast33 = bcast33.astype(ml_dtypes.bfloat16)

    xT = [np.ascontiguousarray(x[b].T) for b in range(B)]  # [D, T]

    in_maps = []
    for c in range(N_CORES):
        b, g = divmod(c, G)
        rows = slice(g * DG, (g + 1) * DG)
        wqT = np.ascontiguousarray((Wq[rows] * scale).T).astype(ml_dtypes.bfloat16)
        wkT = np.ascontiguousarray(Wkv[0:D][rows].T).astype(ml_dtypes.bfloat16)
        wvT = np.ascontiguousarray(Wkv[D : 2 * D][rows].T).astype(ml_dtypes.bfloat16)
        woT = np.ascontiguousarray(Wo[:, rows].T).astype(ml_dtypes.bfloat16)
        qaug = np.zeros((HPC, 3, T), np.float32)
        for h in range(HPC):
            slope = (g * HPC + h + 1) / H
            qaug[h, 0, :] = slope
            qaug[h, 1, :] = slope
            qaug[h, 2, :] = -slope * iota
        m = {
            "xT": xT[b],
            "wqT": wqT,
            "wkT": wkT,
            "wvT": wvT,
            "woT": woT,
            "qaug": qaug.astype(ml_dtypes.bfloat16),
            "kaug": kaug,
            "ident": ident,
            "maskst": maskst,
            "bcast33": bcast33,
        }
        if with_bias:
            bv_g = bkv[D : 2 * D][rows]
            m["bvo"] = np.broadcast_to(bv_g[None, :], (P, DG)).copy()
            m["bq2"] = np.ascontiguousarray((bq[rows] * scale).reshape(2, P).T)
            m["bk2"] = np.ascontiguousarray(bkv[0:D][rows].reshape(2, P).T)
        in_maps.append(m)

    res = run_bass_kernel_spmd(nc, in_maps, list(range(N_CORES)), trace=TRACE)
    global LAST_RESULTS
    LAST_RESULTS = res

    out = np.empty((B, T, D), np.float32)
    for b in range(B):
        acc = res.results[b * G]["outT"].astype(np.float32)
        for g in range(1, G):
            acc += res.results[b * G + g]["outT"].astype(np.float32)
        out[b] = acc.T + bo[None, :]
    return out
